# revision 1
# baseline (speedup 1.0000x reference)
"""MoE top-2 routing kernel for Trainium2 (8 NeuronCores, batch-sharded).

Problem (hardcoded shapes):
    x [8192, 3072] f32, Wg [3072, 8], bg [8], W1 [8, 3072, 128], b1 [8, 128],
    W2 [8, 128, 10], b2 [8, 10]  ->  out [8192, 10] f32
    g = x@Wg + bg; top-2 softmax over selected logits;
    y = sum_k w_k * (relu(x@W1[e_k] + b1[e_k]) @ W2[e_k] + b2[e_k])

Design (per core, 1024 tokens, dense over experts):
  - x tiles are PE-transposed (fp32, exact) to xT chunks [128d, 512t].
  - Gating matmul in full fp32 (top-2 selection must match the fp32
    reference; float32r's 1e-4 logit error would flip near-ties).
  - Expert matmuls in float32r (TF32-like, 1 cycle/row at N=512) with
    fp32 PSUM accumulation -> ~1.5e-4 rel err.
  - Top-2 via DVE sort-8; combine weights built with is_equal masks.
  - Per-expert y [10, 512] transposed back to token-major [128, 10] and
    scaled by per-token weight via tensor_scalar; accumulated on DVE.
"""
import sys

for _p in ("/opt/trn_rl_repo",):
    if _p not in sys.path:
        sys.path.insert(0, _p)

import numpy as np
from contextlib import ExitStack

import concourse.bass as bass
import concourse.bacc as bacc
import concourse.tile as tile
import concourse.mybir as mybir
from concourse import bass_utils, masks

F32 = mybir.dt.float32
F32R = mybir.dt.float32r
AF = mybir.ActivationFunctionType
OP = mybir.AluOpType

B, D, H, O, NE = 8192, 3072, 128, 10, 8
NCORES = 8
BC = B // NCORES          # tokens per core
TT = 512                  # token tile
NT = BC // TT             # token tiles per core
NCH = D // 128            # contraction chunks
NG = TT // 128            # 128-token groups per tile

_CACHE = {}


def _build_program():
    nc = bacc.Bacc("TRN2", target_bir_lowering=False, debug=False,
                   num_devices=NCORES)

    x = nc.dram_tensor("x", [BC, D], F32, kind="ExternalInput").ap()
    wg = nc.dram_tensor("Wg", [D, NE], F32, kind="ExternalInput").ap()
    bg = nc.dram_tensor("bg", [NE], F32, kind="ExternalInput").ap()
    w1 = nc.dram_tensor("W1", [NE, D, H], F32, kind="ExternalInput").ap()
    b1 = nc.dram_tensor("b1", [NE, H], F32, kind="ExternalInput").ap()
    w2 = nc.dram_tensor("W2", [NE, H, O], F32, kind="ExternalInput").ap()
    b2 = nc.dram_tensor("b2", [NE, O], F32, kind="ExternalInput").ap()
    out = nc.dram_tensor("out", [BC, O], F32, kind="ExternalOutput").ap()

    with tile.TileContext(nc) as tc:
        with ExitStack() as ctx:
            _kernel_body(ctx, tc, nc, x, wg, bg, w1, b1, w2, b2, out)
    nc.compile()
    return nc


def _kernel_body(ctx, tc, nc, x, wg, bg, w1, b1, w2, b2, out):
    singles = ctx.enter_context(tc.tile_pool(name="singles", bufs=1))
    w1stage = ctx.enter_context(tc.tile_pool(name="w1stage", bufs=2))
    xin_p = ctx.enter_context(tc.tile_pool(name="xin", bufs=4))
    xtf_p = ctx.enter_context(tc.tile_pool(name="xtf", bufs=3))
    xtr_p = ctx.enter_context(tc.tile_pool(name="xtr", bufs=1))
    gate_p = ctx.enter_context(tc.tile_pool(name="gate", bufs=2))
    hr_p = ctx.enter_context(tc.tile_pool(name="hr", bufs=2))
    yout_p = ctx.enter_context(tc.tile_pool(name="yout", bufs=2))

    ps_xtp = ctx.enter_context(tc.tile_pool(name="ps_xtp", bufs=2, space="PSUM"))
    ps_g = ctx.enter_context(tc.tile_pool(name="ps_g", bufs=1, space="PSUM"))
    ps_h = ctx.enter_context(tc.tile_pool(name="ps_h", bufs=2, space="PSUM"))
    ps_y = ctx.enter_context(tc.tile_pool(name="ps_y", bufs=1, space="PSUM"))
    ps_s = ctx.enter_context(tc.tile_pool(name="ps_s", bufs=2, space="PSUM"))

    # ---- constants ----
    ident = singles.tile([128, 128], F32)
    masks.make_identity(nc, ident[:])

    # constants + weights go on the scalar-engine DGE queue so the x-tile
    # loads on the sync queue are never stuck behind them
    wg_sb = singles.tile([128, NCH, NE], F32)
    nc.gpsimd.dma_start(wg_sb[:], wg.rearrange("(c j) e -> j c e", j=128))
    # gating weights as a float32r hi/lo pair (covers Wg to ~22 mantissa
    # bits, so top-2 selection matches the fp32 reference)
    wg_r = singles.tile([128, NCH, NE], F32R)
    nc.vector.tensor_copy(wg_r[:], wg_sb[:])
    wg_lo = singles.tile([128, NCH, NE], F32R)
    nc.vector.tensor_tensor(
        wg_lo[:], wg_sb[:], wg_r[:].bitcast(F32), op=OP.subtract
    )
    bg_sb = singles.tile([NE, 1], F32)
    nc.gpsimd.dma_start(bg_sb[:], bg.rearrange("(e one) -> e one", one=1))
    b1t_sb = singles.tile([H, NE], F32)
    nc.gpsimd.dma_start(b1t_sb[:], b1.rearrange("e h -> h e"))
    b2t_sb = singles.tile([O, NE], F32)
    nc.gpsimd.dma_start(b2t_sb[:], b2.rearrange("e o -> o e"))

    w2st = singles.tile([H, NE, O], F32)
    nc.gpsimd.dma_start(w2st[:], w2.rearrange("e h o -> h e o"))
    w2_r = singles.tile([H, NE, O], F32R)
    nc.vector.tensor_copy(w2_r[:], w2st[:])

    # ---- W1 resident in float32r (rounding copies on DVE, off ACT) ----
    # one tile per expert so expert e's matmuls only depend on cast e
    w1_r = []
    for e in range(NE):
        st = w1stage.tile([128, NCH, H], F32, tag="w1st")
        nc.gpsimd.dma_start(st[:], w1[e].rearrange("(c j) h -> j c h", j=128))
        w1e = singles.tile([128, NCH, H], F32R, tag=f"w1r{e}")
        for cc in range(NCH):
            nc.vector.tensor_copy(w1e[:, cc], st[:, cc])
        w1_r.append(w1e)

    # ---- per token tile ----
    for t in range(NT):
        tok0 = t * TT

        # per-chunk xT tiles: expert matmuls for chunk c depend only on
        # chunk c's copy, so they can fill chunk-phase PE gaps
        xtrs = [xtr_p.tile([128, TT], F32R, tag=f"xtr{c}", name=f"xtr{c}") for c in range(NCH)]
        g_ps = ps_g.tile([NE, TT], F32, tag="g")

        SKEW = 2  # emit gating for chunk c-SKEW so PE never waits on copies
        NPRE = 2  # experts whose h-accumulation interleaves the chunk loop
        h_pre = [
            ps_h.tile([128, TT], F32, tag="h", name=f"hpre{t}_{e}")
            for e in range(NPRE)
        ]
        xlos = {}

        def gating(cg):
            # (xtr + xlo) @ wg_r + xtr @ wg_lo covers the product to ~2^-22
            # -- top-2 selection is fp32-faithful
            nc.tensor.matmul(
                g_ps[:], wg_r[:, cg, :], xtrs[cg][:],
                start=(cg == 0), stop=False,
            )
            nc.tensor.matmul(
                g_ps[:], wg_r[:, cg, :], xlos.pop(cg)[:], start=False, stop=False
            )
            nc.tensor.matmul(
                g_ps[:], wg_lo[:, cg, :], xtrs[cg][:],
                start=False, stop=(cg == NCH - 1),
            )

        for c in range(NCH):
            xin = xin_p.tile([128, NG, 128], F32, tag="xin")
            nc.sync.dma_start(
                xin[:],
                x[tok0 : tok0 + TT, c * 128 : (c + 1) * 128].rearrange(
                    "(gg p) d -> p gg d", p=128
                ),
            )
            xtp = ps_xtp.tile([128, TT], F32, tag="xtp")
            for gg in range(NG):
                nc.tensor.matmul(
                    xtp[:, gg * 128 : (gg + 1) * 128],
                    xin[:, gg, :],
                    ident[:],
                    is_transpose=True,
                    start=True,
                    stop=True,
                    skip_group_check=True,
                )
            # xtr = round_to_f32r(xT); xlo = round_to_f32r(xT - xtr)
            nc.scalar.copy(xtrs[c][:], xtp[:])
            xlo = xtf_p.tile([128, TT], F32R, tag="xlo")
            nc.vector.tensor_tensor(
                xlo[:], xtp[:], xtrs[c][:].bitcast(F32), op=OP.subtract
            )
            xlos[c] = xlo
            if c >= SKEW:
                gating(c - SKEW)
            # pre-emit the first two experts' accumulation for this chunk:
            # fills chunk-phase PE stalls with useful stream work
            for e in range(NPRE):
                nc.tensor.matmul(
                    h_pre[e][:],
                    w1_r[e][:, c, :],
                    xtrs[c][:],
                    start=(c == 0),
                    stop=(c == NCH - 1),
                )
        for cg in range(NCH - SKEW, NCH):
            gating(cg)

        # ---- gating epilogue: top-2 softmax -> wfull [128, NG*NE] ----
        g_sb = gate_p.tile([NE, TT], F32, tag="gsb")
        nc.vector.tensor_scalar(g_sb[:], g_ps[:], bg_sb[:, 0:1], None, OP.add)

        wfull = gate_p.tile([128, NG * NE], F32, tag="wfull")
        for gg in range(NG):
            gt_ps = ps_s.tile([128, NE], F32, tag="s")
            nc.tensor.transpose(
                gt_ps[:], g_sb[:, gg * 128 : (gg + 1) * 128], ident[0:NE, 0:NE]
            )
            gt = gate_p.tile([128, NE], F32, tag="gt")
            nc.vector.tensor_copy(gt[:], gt_ps[:])

            maxs = gate_p.tile([128, 8], F32, tag="maxs")
            nc.vector.max(maxs[:], gt[:])
            top1, top2 = maxs[:, 0:1], maxs[:, 1:2]

            sm = gate_p.tile([128, 4], F32, tag="sm")
            d21, e21, den, w2c = (sm[:, i : i + 1] for i in range(4))
            nc.vector.tensor_sub(d21, top2, top1)
            nc.scalar.activation(e21, d21, AF.Exp)
            nc.vector.tensor_scalar(den, e21, 1.0, None, OP.add)
            w1c = gate_p.tile([128, 1], F32, tag="w1c")
            nc.vector.reciprocal(w1c[:], den)
            nc.vector.tensor_mul(w2c, e21, w1c[:])

            m1 = gate_p.tile([128, NE], F32, tag="m1")
            m2 = gate_p.tile([128, NE], F32, tag="m2")
            nc.vector.tensor_scalar(m1[:], gt[:], top1, None, OP.is_equal)
            nc.vector.tensor_scalar(m2[:], gt[:], top2, None, OP.is_equal)
            nc.vector.tensor_scalar(m1[:], m1[:], w1c[:, 0:1], None, OP.mult)
            nc.vector.tensor_scalar(m2[:], m2[:], w2c, None, OP.mult)
            nc.vector.tensor_add(
                wfull[:, gg * NE : (gg + 1) * NE], m1[:], m2[:]
            )

        # ---- expert loop ----
        yt_acc = yout_p.tile([128, NG * O], F32, tag="ytacc")
        for e in range(NE):
            if e < NPRE:
                h_ps = h_pre[e]
            else:
                h_ps = ps_h.tile([128, TT], F32, tag="h")
                for c in range(NCH):
                    nc.tensor.matmul(
                        h_ps[:],
                        w1_r[e][:, c, :],
                        xtrs[c][:],
                        start=(c == 0),
                        stop=(c == NCH - 1),
                    )
            hr = hr_p.tile([128, TT], F32R, tag="hr")
            nc.scalar.activation(
                hr[:], h_ps[:], AF.Relu, bias=b1t_sb[:, e : e + 1]
            )
            y_ps = ps_y.tile([O, TT], F32, tag="y")
            nc.tensor.matmul(y_ps[:], w2_r[:, e, :], hr[:], start=True, stop=True)
            y_sb = yout_p.tile([O, TT], F32, tag="ysb")
            nc.vector.tensor_scalar(
                y_sb[:], y_ps[:], b2t_sb[:, e : e + 1], None, OP.add
            )
            for gg in range(NG):
                yt_ps = ps_s.tile([128, O], F32, tag="s")
                nc.tensor.transpose(
                    yt_ps[:],
                    y_sb[:, gg * 128 : (gg + 1) * 128],
                    ident[0:O, 0:O],
                )
                w_col = wfull[:, gg * NE + e : gg * NE + e + 1]
                acc = yt_acc[:, gg * O : (gg + 1) * O]
                if e == 0:
                    nc.vector.tensor_scalar(acc, yt_ps[:], w_col, None, OP.mult)
                else:
                    tmp = yout_p.tile([128, O], F32, tag="yttmp")
                    nc.vector.tensor_scalar(tmp[:], yt_ps[:], w_col, None, OP.mult)
                    nc.vector.tensor_add(acc, acc, tmp[:])

        nc.sync.dma_start(
            out[tok0 : tok0 + TT].rearrange("(gg p) o -> p gg o", p=128),
            yt_acc[:].rearrange("p (gg o) -> p gg o", gg=NG),
        )


def _get_nc():
    if "nc" not in _CACHE:
        _CACHE["nc"] = _build_program()
    return _CACHE["nc"]


def kernel(x, Wg, bg, W1, b1, W2, b2, _trace=False, _tmpdir=None):
    nc = _get_nc()
    x = np.ascontiguousarray(np.asarray(x, dtype=np.float32))
    shared = {
        "Wg": np.ascontiguousarray(np.asarray(Wg, dtype=np.float32)),
        "bg": np.ascontiguousarray(np.asarray(bg, dtype=np.float32)),
        "W1": np.ascontiguousarray(np.asarray(W1, dtype=np.float32)),
        "b1": np.ascontiguousarray(np.asarray(b1, dtype=np.float32)),
        "W2": np.ascontiguousarray(np.asarray(W2, dtype=np.float32)),
        "b2": np.ascontiguousarray(np.asarray(b2, dtype=np.float32)),
    }
    in_maps = [
        {"x": x[c * BC : (c + 1) * BC], **shared} for c in range(NCORES)
    ]
    res = bass_utils.run_bass_kernel_spmd(
        nc,
        in_maps,
        core_ids=list(range(NCORES)),
        trace=_trace,
        tmpdir=_tmpdir,
    )
    outp = np.concatenate([res.results[c]["out"] for c in range(NCORES)], axis=0)
    if _trace:
        kernel._last_results = res
    return outp



# revision 14
# speedup vs baseline: 1.0120x; 1.0120x over previous
"""MoE top-2 routing kernel for Trainium2 (8 NeuronCores, batch-sharded).

Problem (hardcoded shapes):
    x [8192, 3072] f32, Wg [3072, 8], bg [8], W1 [8, 3072, 128], b1 [8, 128],
    W2 [8, 128, 10], b2 [8, 10]  ->  out [8192, 10] f32
    g = x@Wg + bg; top-2 softmax over selected logits;
    y = sum_k w_k * (relu(x@W1[e_k] + b1[e_k]) @ W2[e_k] + b2[e_k])

v2 design (per core, 1024 tokens = 2 tiles x 512, dense over experts):
  - PE kept saturated; fp32 transposes (exact), f32r matmuls.
  - Weights arrive as f32r via gpsimd casting DMAs (no stage, no DVE
    cast pass; BIR verifier requires rounded f32r matmul operands).
  - 2-pass gating: xtr@Wg + xlo@Wg where xtr = f32r-round(xT) (ACT
    PSUM->SBUF copy) and xlo = xT - xtr (DVE).  Measured 5.6e-3 rel err.
  - Experts in 8 single-expert waves per tile: wave 1 carries the
    transposes + gating; waves 2..8 are pure h-accumulation; expert e's
    combine (relu -> *w -> W2 matmul into shared y PSUM) is injected a
    few chunks into wave e+1 to hide ACT/gpsimd/DVE latency.
  - Top-2 epilogue in [8, 512] orientation: partition_all_reduce(max)
    for m1/m2, is_equal masks, softmax weights -> wT [8, 512]; no PE.
  - Combine weights: wbc_e = partition_broadcast(wT[e]) (gpsimd),
    hrw = hr*wbc (DVE, f32r out), y_ps += W2_e^T @ hrw_e (+ b2^T @ wT).
  - DMA: x chunks on sync queue; W1/consts casting DMAs on gpsimd
    (SWDGE); out on scalar queue.
"""
import sys

for _p in ("/opt/trn_rl_repo",):
    if _p not in sys.path:
        sys.path.insert(0, _p)

import numpy as np
from contextlib import ExitStack

import concourse.bass as bass
import concourse.bacc as bacc
import concourse.bass_isa as bass_isa
import concourse.tile as tile
import concourse.mybir as mybir
from concourse import bass_utils, masks

F32 = mybir.dt.float32
F32R = mybir.dt.float32r
AF = mybir.ActivationFunctionType
OP = mybir.AluOpType

B, D, H, O, NE = 8192, 3072, 128, 10, 8
NCORES = 8
BC = B // NCORES          # tokens per core
TT = 512                  # token tile
NT = BC // TT             # token tiles per core
NCH = D // 128            # contraction chunks
NG = TT // 128            # 128-token groups per tile

_CACHE = {}


def _build_program():
    nc = bacc.Bacc("TRN2", target_bir_lowering=False, debug=False,
                   num_devices=NCORES)

    x = nc.dram_tensor("x", [BC, D], F32, kind="ExternalInput").ap()
    wg = nc.dram_tensor("Wg", [D, NE], F32, kind="ExternalInput").ap()
    bg = nc.dram_tensor("bg", [NE], F32, kind="ExternalInput").ap()
    w1 = nc.dram_tensor("W1", [NE, D, H], F32, kind="ExternalInput").ap()
    b1 = nc.dram_tensor("b1", [NE, H], F32, kind="ExternalInput").ap()
    w2 = nc.dram_tensor("W2", [NE, H, O], F32, kind="ExternalInput").ap()
    b2 = nc.dram_tensor("b2", [NE, O], F32, kind="ExternalInput").ap()
    out = nc.dram_tensor("out", [BC, O], F32, kind="ExternalOutput").ap()

    with tile.TileContext(nc) as tc:
        with ExitStack() as ctx:
            _kernel_body(ctx, tc, nc, x, wg, bg, w1, b1, w2, b2, out)
    nc.compile()
    return nc


def _kernel_body(ctx, tc, nc, x, wg, bg, w1, b1, w2, b2, out):
    singles = ctx.enter_context(tc.tile_pool(name="singles", bufs=1))
    xin_p = ctx.enter_context(tc.tile_pool(name="xin", bufs=3))
    xtr_p = ctx.enter_context(tc.tile_pool(name="xtr", bufs=NCH))
    xlo_p = ctx.enter_context(tc.tile_pool(name="xlo", bufs=3))
    gate_p = ctx.enter_context(tc.tile_pool(name="gate", bufs=8))
    wt_p = ctx.enter_context(tc.tile_pool(name="wt", bufs=2))
    hr_p = ctx.enter_context(tc.tile_pool(name="hr", bufs=2))
    hrw_p = ctx.enter_context(tc.tile_pool(name="hrw", bufs=2))
    wrow_p = ctx.enter_context(tc.tile_pool(name="wrow", bufs=2))
    wrowr_p = ctx.enter_context(tc.tile_pool(name="wrowr", bufs=2))
    yout_p = ctx.enter_context(tc.tile_pool(name="yout", bufs=2))

    ps_xtp = ctx.enter_context(tc.tile_pool(name="ps_xtp", bufs=2, space="PSUM"))
    ps_g = ctx.enter_context(tc.tile_pool(name="ps_g", bufs=1, space="PSUM"))
    ps_h = ctx.enter_context(tc.tile_pool(name="ps_h", bufs=3, space="PSUM"))
    ps_y = ctx.enter_context(tc.tile_pool(name="ps_y", bufs=1, space="PSUM"))
    ps_wb = ctx.enter_context(tc.tile_pool(name="ps_wb", bufs=1, space="PSUM"))

    # ---- constants ----
    ident = singles.tile([128, 128], F32)
    masks.make_identity(nc, ident[:])
    ones_f = singles.tile([1, 128], F32)
    nc.vector.memset(ones_f[:], 1.0)
    ones_r = singles.tile([1, 128], F32R)
    nc.vector.tensor_copy(ones_r[:], ones_f[:])

    bg_sb = singles.tile([NE, 1], F32)
    nc.scalar.dma_start(bg_sb[:], bg.rearrange("(e one) -> e one", one=1))
    b1t_sb = singles.tile([H, NE], F32)
    nc.scalar.dma_start(b1t_sb[:], b1.rearrange("e h -> h e"))

    # f32r weights via gpsimd casting DMAs (SWDGE converts f32 -> f32r)
    wg_r = singles.tile([128, NCH, NE], F32R)
    nc.gpsimd.dma_start(wg_r[:], wg.rearrange("(c j) e -> j c e", j=128))
    w2_r = singles.tile([H, NE, O], F32R)
    nc.gpsimd.dma_start(w2_r[:], w2.rearrange("e h o -> h e o"))
    b2_r = singles.tile([NE, O], F32R)
    nc.gpsimd.dma_start(b2_r[:], b2)
    w1_r = []
    for e in range(NE):
        w1e = singles.tile([128, NCH, H], F32R, tag=f"w1_{e}", name=f"w1_{e}")
        nc.gpsimd.dma_start(w1e[:], w1[e].rearrange("(c j) h -> j c h", j=128))
        w1_r.append(w1e)

    # ---- per token tile ----
    for t in range(NT):
        tok0 = t * TT

        xtrs = [
            xtr_p.tile([128, TT], F32R, tag="xtr", name=f"xtr{t}_{c}")
            for c in range(NCH)
        ]
        xlos = {}
        g_ps = ps_g.tile([NE, TT], F32, tag="g")

        def transpose_chunk(c):
            xin = xin_p.tile([128, NG, 128], F32, tag="xin")
            nc.sync.dma_start(
                xin[:],
                x[tok0 : tok0 + TT, c * 128 : (c + 1) * 128].rearrange(
                    "(gg p) d -> p gg d", p=128
                ),
            )
            xtp = ps_xtp.tile([128, TT], F32, tag="xtp")
            for gg in range(NG):
                nc.tensor.matmul(
                    xtp[:, gg * 128 : (gg + 1) * 128],
                    xin[:, gg, :],
                    ident[:],
                    is_transpose=True,
                    start=True,
                    stop=True,
                    skip_group_check=True,
                )
            # xtr = f32r-round(xT) on ACT; xlo = xT - xtr on DVE
            nc.scalar.copy(xtrs[c][:], xtp[:])
            xlo = xlo_p.tile([128, TT], F32R, tag="xlo")
            nc.vector.tensor_tensor(
                xlo[:], xtp[:], xtrs[c][:].bitcast(F32), op=OP.subtract
            )
            xlos[c] = xlo

        def gating(c):
            nc.tensor.matmul(
                g_ps[:], wg_r[:, c, :], xtrs[c][:],
                start=(c == 0), stop=False,
            )
            nc.tensor.matmul(
                g_ps[:], wg_r[:, c, :], xlos.pop(c)[:],
                start=False, stop=(c == NCH - 1),
            )

        def h_matmul(e, h_ps, c):
            nc.tensor.matmul(
                h_ps[:],
                w1_r[e][:, c],
                xtrs[c][:],
                start=(c == 0),
                stop=(c == NCH - 1),
            )

        # ---- wave 1 (expert 0) carries transposes + gating ----
        h_cur = ps_h.tile([128, TT], F32, tag="h", name=f"h{t}_0")
        for c in range(NCH):
            transpose_chunk(c)
            if c >= 2:
                gating(c - 2)
            if c >= 1:
                h_matmul(0, h_cur, c - 1)
        gating(NCH - 2)
        gating(NCH - 1)
        h_matmul(0, h_cur, NCH - 1)

        # ---- gating epilogue (DVE/ACT/gpsimd; no PE) -> wT [8, TT] ----
        g_sb = gate_p.tile([NE, TT], F32, tag="ge")
        nc.vector.tensor_scalar(g_sb[:], g_ps[:], bg_sb[:, 0:1], None, OP.add)
        m1 = gate_p.tile([NE, TT], F32, tag="ge")
        nc.gpsimd.partition_all_reduce(
            m1[:], g_sb[:], channels=NE, reduce_op=bass_isa.ReduceOp.max
        )
        eq1 = gate_p.tile([NE, TT], F32, tag="ge")
        nc.vector.tensor_tensor(eq1[:], g_sb[:], m1[:], op=OP.is_equal)
        negb = gate_p.tile([NE, TT], F32, tag="ge")
        nc.vector.tensor_scalar(negb[:], eq1[:], -1e30, None, OP.mult)
        g2 = gate_p.tile([NE, TT], F32, tag="ge")
        nc.vector.tensor_tensor(g2[:], g_sb[:], negb[:], op=OP.add)
        m2 = gate_p.tile([NE, TT], F32, tag="ge")
        nc.gpsimd.partition_all_reduce(
            m2[:], g2[:], channels=NE, reduce_op=bass_isa.ReduceOp.max
        )
        eq2 = gate_p.tile([NE, TT], F32, tag="ge")
        nc.vector.tensor_tensor(eq2[:], g_sb[:], m2[:], op=OP.is_equal)
        d21 = gate_p.tile([NE, TT], F32, tag="ge")
        nc.vector.tensor_tensor(d21[:], m2[:], m1[:], op=OP.subtract)
        ex = gate_p.tile([NE, TT], F32, tag="ge")
        nc.scalar.activation(ex[:], d21[:], AF.Exp)
        den = gate_p.tile([NE, TT], F32, tag="ge")
        nc.vector.tensor_scalar(den[:], ex[:], 1.0, None, OP.add)
        rcp = gate_p.tile([NE, TT], F32, tag="ge")
        nc.vector.reciprocal(rcp[:], den[:])
        t1 = gate_p.tile([NE, TT], F32, tag="ge")
        nc.vector.tensor_tensor(t1[:], eq1[:], rcp[:], op=OP.mult)
        w2v = gate_p.tile([NE, TT], F32, tag="ge")
        nc.vector.tensor_tensor(w2v[:], ex[:], rcp[:], op=OP.mult)
        t2 = gate_p.tile([NE, TT], F32, tag="ge")
        nc.vector.tensor_tensor(t2[:], eq2[:], w2v[:], op=OP.mult)
        wT = wt_p.tile([NE, TT], F32R, tag="wt")
        nc.vector.tensor_tensor(wT[:], t1[:], t2[:], op=OP.add)

        y_ps = ps_y.tile([O, TT], F32, tag="y")
        n_acc = [0]

        def relu_expert(e, h_ps):
            hr = hr_p.tile([128, TT], F32R, tag="hr", name=f"hr{t}_{e}")
            nc.scalar.activation(
                hr[:], h_ps[:], AF.Relu, bias=b1t_sb[:, e : e + 1]
            )
            return hr

        def combine_expert(e, hr):
            # wT[e] row -> partition 0 (DMA remap), round, then broadcast
            # across partitions via PE rank-1 outer product
            wrow = wrow_p.tile([1, TT], F32, tag="wrow", name=f"wrow{t}_{e}")
            nc.scalar.dma_start(wrow[:], wT[e : e + 1, :].bitcast(F32))
            wrow_r = wrowr_p.tile([1, TT], F32R, tag="wrowr", name=f"wrowr{t}_{e}")
            nc.vector.tensor_copy(wrow_r[:], wrow[:])
            wbc = ps_wb.tile([128, TT], F32, tag="wbc", name=f"wbc{t}_{e}")
            nc.tensor.matmul(
                wbc[:], ones_r[:], wrow_r[:], start=True, stop=True
            )
            hrw = hrw_p.tile([128, TT], F32R, tag="hrw", name=f"hrw{t}_{e}")
            nc.vector.tensor_tensor(
                hrw[:], hr[:].bitcast(F32), wbc[:], op=OP.mult
            )
            nc.tensor.matmul(
                y_ps[:],
                w2_r[:, e, :],
                hrw[:],
                start=(n_acc[0] == 0),
                stop=False,
            )
            n_acc[0] += 1

        # ---- waves 2..8 (experts 1..7); combine(e-1) injected into wave e
        pend = (0, relu_expert(0, h_cur), h_cur)
        for e in range(1, NE):
            h_nxt = ps_h.tile([128, TT], F32, tag="h", name=f"h{t}_{e}")
            for c in range(NCH):
                h_matmul(e, h_nxt, c)
                if c == 10 and pend is not None:
                    combine_expert(pend[0], pend[1])
                    pend = None
            pend = (e, relu_expert(e, h_nxt), h_nxt)
        combine_expert(pend[0], pend[1])

        # b2 contribution: sum_e wT[e,t] * b2[e,:]
        nc.tensor.matmul(
            y_ps[:], b2_r[:], wT[:], start=False, stop=True,
        )

        # ---- output: [O, TT] -> token-major [TT, O] ----
        y_sb = yout_p.tile([O, TT], F32, tag="ysb")
        nc.vector.tensor_copy(y_sb[:], y_ps[:])
        # ride the xtp slot ring (same [128, TT] f32 shape) — no extra bank
        yt_ps = ps_xtp.tile([128, TT], F32, tag="xtp", name=f"ytps{t}")
        for gg in range(NG):
            nc.tensor.matmul(
                yt_ps[:, gg * O : (gg + 1) * O],
                y_sb[:, gg * 128 : (gg + 1) * 128],
                ident[0:O, 0:O],
                is_transpose=True,
                start=True,
                stop=True,
                skip_group_check=True,
            )
        yt_sb = yout_p.tile([128, NG * O], F32, tag="ytsb")
        nc.vector.tensor_copy(yt_sb[:], yt_ps[:, 0 : NG * O])
        nc.scalar.dma_start(
            out[tok0 : tok0 + TT].rearrange("(gg p) o -> p gg o", p=128),
            yt_sb[:].rearrange("p (gg o) -> p gg o", gg=NG),
        )


def _get_nc():
    if "nc" not in _CACHE:
        _CACHE["nc"] = _build_program()
    return _CACHE["nc"]


def kernel(x, Wg, bg, W1, b1, W2, b2, _trace=False, _tmpdir=None):
    nc = _get_nc()
    x = np.ascontiguousarray(np.asarray(x, dtype=np.float32))
    shared = {
        "Wg": np.ascontiguousarray(np.asarray(Wg, dtype=np.float32)),
        "bg": np.ascontiguousarray(np.asarray(bg, dtype=np.float32)),
        "W1": np.ascontiguousarray(np.asarray(W1, dtype=np.float32)),
        "b1": np.ascontiguousarray(np.asarray(b1, dtype=np.float32)),
        "W2": np.ascontiguousarray(np.asarray(W2, dtype=np.float32)),
        "b2": np.ascontiguousarray(np.asarray(b2, dtype=np.float32)),
    }
    in_maps = [
        {"x": x[c * BC : (c + 1) * BC], **shared} for c in range(NCORES)
    ]
    res = bass_utils.run_bass_kernel_spmd(
        nc,
        in_maps,
        core_ids=list(range(NCORES)),
        trace=_trace,
        tmpdir=_tmpdir,
    )
    outp = np.concatenate([res.results[c]["out"] for c in range(NCORES)], axis=0)
    if _trace:
        kernel._last_results = res
    return outp


# revision 18
# speedup vs baseline: 1.3472x; 1.3313x over previous
"""MoE top-2 routing kernel for Trainium2 (8 NeuronCores, batch-sharded).

Problem (hardcoded shapes):
    x [8192, 3072] f32, Wg [3072, 8], bg [8], W1 [8, 3072, 128], b1 [8, 128],
    W2 [8, 128, 10], b2 [8, 10]  ->  out [8192, 10] f32
    g = x@Wg + bg; top-2 softmax over selected logits;
    y = sum_k w_k * (relu(x@W1[e_k] + b1[e_k]) @ W2[e_k] + b2[e_k])

v2 design (per core, 1024 tokens = 2 tiles x 512, dense over experts):
  - PE kept saturated; fp32 transposes (exact), f32r matmuls.
  - Weights arrive as f32r via gpsimd casting DMAs (no stage, no DVE
    cast pass; BIR verifier requires rounded f32r matmul operands).
  - 2-pass gating: xtr@Wg + xlo@Wg where xtr = f32r-round(xT) (ACT
    PSUM->SBUF copy) and xlo = xT - xtr (DVE).  Measured 5.6e-3 rel err.
  - Experts in 8 single-expert waves per tile: wave 1 carries the
    transposes + gating; waves 2..8 are pure h-accumulation; expert e's
    combine (relu -> *w -> W2 matmul into shared y PSUM) is injected a
    few chunks into wave e+1 to hide ACT/gpsimd/DVE latency.
  - Top-2 epilogue in [8, 512] orientation: partition_all_reduce(max)
    for m1/m2, is_equal masks, softmax weights -> wT [8, 512]; no PE.
  - Combine weights: wbc_e = partition_broadcast(wT[e]) (gpsimd),
    hrw = hr*wbc (DVE, f32r out), y_ps += W2_e^T @ hrw_e (+ b2^T @ wT).
  - DMA: x chunks on sync queue; W1/consts casting DMAs on gpsimd
    (SWDGE); out on scalar queue.
"""
import sys

for _p in ("/opt/trn_rl_repo",):
    if _p not in sys.path:
        sys.path.insert(0, _p)

import numpy as np
from contextlib import ExitStack

import concourse.bass as bass
import concourse.bacc as bacc
import concourse.bass_isa as bass_isa
import concourse.tile as tile
import concourse.mybir as mybir
from concourse import bass_utils, masks

F32 = mybir.dt.float32
F32R = mybir.dt.float32r
AF = mybir.ActivationFunctionType
OP = mybir.AluOpType

B, D, H, O, NE = 8192, 3072, 128, 10, 8
NCORES = 8
BC = B // NCORES          # tokens per core
TT = 512                  # token tile
NT = BC // TT             # token tiles per core
NCH = D // 128            # contraction chunks
NG = TT // 128            # 128-token groups per tile

_CACHE = {}


def _build_program():
    nc = bacc.Bacc("TRN2", target_bir_lowering=False, debug=False,
                   num_devices=NCORES)

    x = nc.dram_tensor("x", [BC, D], F32, kind="ExternalInput").ap()
    wg = nc.dram_tensor("Wg", [D, NE], F32, kind="ExternalInput").ap()
    bg = nc.dram_tensor("bg", [NE], F32, kind="ExternalInput").ap()
    w1 = nc.dram_tensor("W1", [NE, D, H], F32, kind="ExternalInput").ap()
    b1 = nc.dram_tensor("b1", [NE, H], F32, kind="ExternalInput").ap()
    w2 = nc.dram_tensor("W2", [NE, H, O], F32, kind="ExternalInput").ap()
    b2 = nc.dram_tensor("b2", [NE, O], F32, kind="ExternalInput").ap()
    out = nc.dram_tensor("out", [BC, O], F32, kind="ExternalOutput").ap()

    with tile.TileContext(nc) as tc:
        with ExitStack() as ctx:
            _kernel_body(ctx, tc, nc, x, wg, bg, w1, b1, w2, b2, out)
    nc.compile()
    return nc


def _kernel_body(ctx, tc, nc, x, wg, bg, w1, b1, w2, b2, out):
    singles = ctx.enter_context(tc.tile_pool(name="singles", bufs=1))
    xin_p = ctx.enter_context(tc.tile_pool(name="xin", bufs=2))
    xtr_p = ctx.enter_context(tc.tile_pool(name="xtr", bufs=NCH))
    xlo_p = ctx.enter_context(tc.tile_pool(name="xlo", bufs=2))
    gate_p = ctx.enter_context(tc.tile_pool(name="gate", bufs=8))
    wt_p = ctx.enter_context(tc.tile_pool(name="wt", bufs=2))
    hr_p = ctx.enter_context(tc.tile_pool(name="hr", bufs=2))
    hrw_p = ctx.enter_context(tc.tile_pool(name="hrw", bufs=1))
    wrow_p = ctx.enter_context(tc.tile_pool(name="wrow", bufs=2))
    wrowr_p = ctx.enter_context(tc.tile_pool(name="wrowr", bufs=2))
    yout_p = ctx.enter_context(tc.tile_pool(name="yout", bufs=2))

    ps_xtp = ctx.enter_context(tc.tile_pool(name="ps_xtp", bufs=2, space="PSUM"))
    ps_g = ctx.enter_context(tc.tile_pool(name="ps_g", bufs=1, space="PSUM"))
    ps_h = ctx.enter_context(tc.tile_pool(name="ps_h", bufs=3, space="PSUM"))
    ps_y = ctx.enter_context(tc.tile_pool(name="ps_y", bufs=1, space="PSUM"))
    ps_wb = ctx.enter_context(tc.tile_pool(name="ps_wb", bufs=1, space="PSUM"))

    # ---- constants ----
    ident = singles.tile([128, 128], F32)
    masks.make_identity(nc, ident[:])
    ones_f = singles.tile([1, 128], F32)
    nc.vector.memset(ones_f[:], 1.0)
    ones_r = singles.tile([1, 128], F32R)
    nc.vector.tensor_copy(ones_r[:], ones_f[:])

    bg_sb = singles.tile([NE, 1], F32)
    nc.scalar.dma_start(bg_sb[:], bg.rearrange("(e one) -> e one", one=1))
    b1t_sb = singles.tile([H, NE], F32)
    nc.scalar.dma_start(b1t_sb[:], b1.rearrange("e h -> h e"))

    # f32r weights via gpsimd casting DMAs (SWDGE converts f32 -> f32r)
    wg_r = singles.tile([128, NCH, NE], F32R)
    nc.gpsimd.dma_start(wg_r[:], wg.rearrange("(c j) e -> j c e", j=128))
    w2_r = singles.tile([H, NE, O], F32R)
    nc.gpsimd.dma_start(w2_r[:], w2.rearrange("e h o -> h e o"))
    b2_r = singles.tile([NE, O], F32R)
    nc.gpsimd.dma_start(b2_r[:], b2)
    w1_r = []
    for e in range(NE):
        w1e = singles.tile([128, NCH, H], F32R, tag=f"w1_{e}", name=f"w1_{e}")
        nc.gpsimd.dma_start(w1e[:], w1[e].rearrange("(c j) h -> j c h", j=128))
        w1_r.append(w1e)

    # ---- per token tile ----
    for t in range(NT):
        tok0 = t * TT

        xtrs = [
            xtr_p.tile([128, TT], F32R, tag="xtr", name=f"xtr{t}_{c}")
            for c in range(NCH)
        ]
        xlos = {}
        g_ps = ps_g.tile([NE, TT], F32, tag="g")

        xblks = {}

        def transpose_chunk(c):
            # x arrives in 4-chunk blocks (2 KB per-partition runs -> near
            # peak DMA efficiency); transposes slice the block per chunk
            XB = 4
            b, ci = divmod(c, XB)
            if ci == 0:
                xin = xin_p.tile([128, NG, XB * 128], F32, tag="xin")
                nc.sync.dma_start(
                    xin[:],
                    x[
                        tok0 : tok0 + TT, b * XB * 128 : (b + 1) * XB * 128
                    ].rearrange("(gg p) d -> p gg d", p=128),
                )
                xblks[b] = xin
            xin = xblks[b]
            xtp = ps_xtp.tile([128, TT], F32, tag="xtp")
            for gg in range(NG):
                nc.tensor.matmul(
                    xtp[:, gg * 128 : (gg + 1) * 128],
                    xin[:, gg, ci * 128 : (ci + 1) * 128],
                    ident[:],
                    is_transpose=True,
                    start=True,
                    stop=True,
                    skip_group_check=True,
                )
            # xtr = f32r-round(xT) on ACT; xlo = xT - xtr on DVE
            nc.scalar.copy(xtrs[c][:], xtp[:])
            xlo = xlo_p.tile([128, TT], F32R, tag="xlo")
            nc.vector.tensor_tensor(
                xlo[:], xtp[:], xtrs[c][:].bitcast(F32), op=OP.subtract
            )
            xlos[c] = xlo

        def gating(c):
            nc.tensor.matmul(
                g_ps[:], wg_r[:, c, :], xtrs[c][:],
                start=(c == 0), stop=False,
            )
            nc.tensor.matmul(
                g_ps[:], wg_r[:, c, :], xlos.pop(c)[:],
                start=False, stop=(c == NCH - 1),
            )

        def h_matmul(e, h_ps, c):
            nc.tensor.matmul(
                h_ps[:],
                w1_r[e][:, c],
                xtrs[c][:],
                start=(c == 0),
                stop=(c == NCH - 1),
            )

        # ---- wave 1 (expert 0) carries transposes + gating ----
        h_cur = ps_h.tile([128, TT], F32, tag="h", name=f"h{t}_0")
        for c in range(NCH):
            transpose_chunk(c)
            if c >= 2:
                gating(c - 2)
            if c >= 1:
                h_matmul(0, h_cur, c - 1)
        gating(NCH - 2)
        gating(NCH - 1)
        h_matmul(0, h_cur, NCH - 1)

        # ---- gating epilogue (DVE/ACT/gpsimd; no PE) -> wT [8, TT] ----
        g_sb = gate_p.tile([NE, TT], F32, tag="ge")
        nc.vector.tensor_scalar(g_sb[:], g_ps[:], bg_sb[:, 0:1], None, OP.add)
        m1 = gate_p.tile([NE, TT], F32, tag="ge")
        nc.gpsimd.partition_all_reduce(
            m1[:], g_sb[:], channels=NE, reduce_op=bass_isa.ReduceOp.max
        )
        eq1 = gate_p.tile([NE, TT], F32, tag="ge")
        nc.vector.tensor_tensor(eq1[:], g_sb[:], m1[:], op=OP.is_equal)
        negb = gate_p.tile([NE, TT], F32, tag="ge")
        nc.vector.tensor_scalar(negb[:], eq1[:], -1e30, None, OP.mult)
        g2 = gate_p.tile([NE, TT], F32, tag="ge")
        nc.vector.tensor_tensor(g2[:], g_sb[:], negb[:], op=OP.add)
        m2 = gate_p.tile([NE, TT], F32, tag="ge")
        nc.gpsimd.partition_all_reduce(
            m2[:], g2[:], channels=NE, reduce_op=bass_isa.ReduceOp.max
        )
        eq2 = gate_p.tile([NE, TT], F32, tag="ge")
        nc.vector.tensor_tensor(eq2[:], g_sb[:], m2[:], op=OP.is_equal)
        d21 = gate_p.tile([NE, TT], F32, tag="ge")
        nc.vector.tensor_tensor(d21[:], m2[:], m1[:], op=OP.subtract)
        ex = gate_p.tile([NE, TT], F32, tag="ge")
        nc.scalar.activation(ex[:], d21[:], AF.Exp)
        den = gate_p.tile([NE, TT], F32, tag="ge")
        nc.vector.tensor_scalar(den[:], ex[:], 1.0, None, OP.add)
        rcp = gate_p.tile([NE, TT], F32, tag="ge")
        nc.vector.reciprocal(rcp[:], den[:])
        t1 = gate_p.tile([NE, TT], F32, tag="ge")
        nc.vector.tensor_tensor(t1[:], eq1[:], rcp[:], op=OP.mult)
        w2v = gate_p.tile([NE, TT], F32, tag="ge")
        nc.vector.tensor_tensor(w2v[:], ex[:], rcp[:], op=OP.mult)
        t2 = gate_p.tile([NE, TT], F32, tag="ge")
        nc.vector.tensor_tensor(t2[:], eq2[:], w2v[:], op=OP.mult)
        wT = wt_p.tile([NE, TT], F32R, tag="wt")
        nc.vector.tensor_tensor(wT[:], t1[:], t2[:], op=OP.add)

        y_ps = ps_y.tile([O, TT], F32, tag="y")
        n_acc = [0]

        def relu_expert(e, h_ps):
            hr = hr_p.tile([128, TT], F32R, tag="hr", name=f"hr{t}_{e}")
            nc.scalar.activation(
                hr[:], h_ps[:], AF.Relu, bias=b1t_sb[:, e : e + 1]
            )
            return hr

        def combine_expert(e, hr):
            # wT[e] row -> partition 0 (DMA remap), round, then broadcast
            # across partitions via PE rank-1 outer product
            wrow = wrow_p.tile([1, TT], F32, tag="wrow", name=f"wrow{t}_{e}")
            nc.scalar.dma_start(wrow[:], wT[e : e + 1, :].bitcast(F32))
            wrow_r = wrowr_p.tile([1, TT], F32R, tag="wrowr", name=f"wrowr{t}_{e}")
            nc.vector.tensor_copy(wrow_r[:], wrow[:])
            wbc = ps_wb.tile([128, TT], F32, tag="wbc", name=f"wbc{t}_{e}")
            nc.tensor.matmul(
                wbc[:], ones_r[:], wrow_r[:], start=True, stop=True
            )
            hrw = hrw_p.tile([128, TT], F32R, tag="hrw", name=f"hrw{t}_{e}")
            nc.vector.tensor_tensor(
                hrw[:], hr[:].bitcast(F32), wbc[:], op=OP.mult
            )
            nc.tensor.matmul(
                y_ps[:],
                w2_r[:, e, :],
                hrw[:],
                start=(n_acc[0] == 0),
                stop=False,
            )
            n_acc[0] += 1

        # ---- waves 2..8 (experts 1..7); combine(e-1) injected into wave e
        pend = (0, relu_expert(0, h_cur), h_cur)
        for e in range(1, NE):
            h_nxt = ps_h.tile([128, TT], F32, tag="h", name=f"h{t}_{e}")
            for c in range(NCH):
                h_matmul(e, h_nxt, c)
                if c == 10 and pend is not None:
                    combine_expert(pend[0], pend[1])
                    pend = None
            pend = (e, relu_expert(e, h_nxt), h_nxt)
        combine_expert(pend[0], pend[1])

        # b2 contribution: sum_e wT[e,t] * b2[e,:]
        nc.tensor.matmul(
            y_ps[:], b2_r[:], wT[:], start=False, stop=True,
        )

        # ---- output: [O, TT] -> token-major [TT, O] ----
        y_sb = yout_p.tile([O, TT], F32, tag="ysb")
        nc.vector.tensor_copy(y_sb[:], y_ps[:])
        # ride the xtp slot ring (same [128, TT] f32 shape) — no extra bank
        yt_ps = ps_xtp.tile([128, TT], F32, tag="xtp", name=f"ytps{t}")
        for gg in range(NG):
            nc.tensor.matmul(
                yt_ps[:, gg * O : (gg + 1) * O],
                y_sb[:, gg * 128 : (gg + 1) * 128],
                ident[0:O, 0:O],
                is_transpose=True,
                start=True,
                stop=True,
                skip_group_check=True,
            )
        yt_sb = yout_p.tile([128, NG * O], F32, tag="ytsb")
        nc.vector.tensor_copy(yt_sb[:], yt_ps[:, 0 : NG * O])
        nc.scalar.dma_start(
            out[tok0 : tok0 + TT].rearrange("(gg p) o -> p gg o", p=128),
            yt_sb[:].rearrange("p (gg o) -> p gg o", gg=NG),
        )


def _get_nc():
    if "nc" not in _CACHE:
        _CACHE["nc"] = _build_program()
    return _CACHE["nc"]


def kernel(x, Wg, bg, W1, b1, W2, b2, _trace=False, _tmpdir=None):
    nc = _get_nc()
    x = np.ascontiguousarray(np.asarray(x, dtype=np.float32))
    shared = {
        "Wg": np.ascontiguousarray(np.asarray(Wg, dtype=np.float32)),
        "bg": np.ascontiguousarray(np.asarray(bg, dtype=np.float32)),
        "W1": np.ascontiguousarray(np.asarray(W1, dtype=np.float32)),
        "b1": np.ascontiguousarray(np.asarray(b1, dtype=np.float32)),
        "W2": np.ascontiguousarray(np.asarray(W2, dtype=np.float32)),
        "b2": np.ascontiguousarray(np.asarray(b2, dtype=np.float32)),
    }
    in_maps = [
        {"x": x[c * BC : (c + 1) * BC], **shared} for c in range(NCORES)
    ]
    res = bass_utils.run_bass_kernel_spmd(
        nc,
        in_maps,
        core_ids=list(range(NCORES)),
        trace=_trace,
        tmpdir=_tmpdir,
    )
    outp = np.concatenate([res.results[c]["out"] for c in range(NCORES)], axis=0)
    if _trace:
        kernel._last_results = res
    return outp
